# revision 1
# baseline (speedup 1.0000x reference)
"""Trainium2 Bass kernel: causal attention with weight-normed QKV projections.

Problem (hardcoded): B=8, Cq=Ck=256, C=512, H=W=32 -> S=1024, N_HEAD=8, dh=64.
Sharding: pure data-parallel over batch (8 batches -> 8 cores), weights
replicated. No collectives.

Host-side glue packs the inputs into 5 merged per-core DMA payloads (the
per-dma_start sequencer issue cost is ~0.7us, so many small DMAs serialize
the startup): vw = [vq;vk;vv] fp32, qk = [q;k] bf16, gb = [gq;gk;gv;bq;bk]
fp32, bv fp32, and msk = [I | strict-upper-ones] bf16 constants.

Per-core pipeline (batch b):
  1. Weight prep: w^T = v.T via PE transpose (identity rhs) in bf16. The
     weight-norm scale g/||v_row|| (ACT Square+accum, fp32) is folded into
     the projection epilogue for Q/K (per-partition there); for V it is
     baked into the transpose (diag(scale) rhs) since V's output layout puts
     channels on the free dim.
  2. Fully interleaved main phase, one C-tile (= one head pair) at a time:
     project QT[ct]/KT[ct] (bf16 operands, fp32 accum, fused scale+bias
     epilogue), then run that pair's QK+exp tile loop with the previous
     pair's PV matmuls interleaved so the PE never sits behind ACT's exp.
     V ([S, C] layout, bias folded via partition-broadcast add, ones column
     per head -> PV also yields softmax denominators) is emitted after ct=0.
  3. Attention per head pair: K=64 QK matmuls row-group packed (the two
     heads live at partitions 0-63 / 64-127 of their C-tile), per-head
     [128, nj] logit psum tiles, exp per head straight out of PSUM into a
     shared bf16 e-tile. Strictly-causal: only the lower triangle of
     [128,128] tiles is computed; diagonal tiles get a strictly-upper bf16
     mask after exp. No max subtraction: logits are O(10).
     PV: out[SqTile, 65] accumulated over S_k tiles with e^T slices as the
     stationary operand (no transposes anywhere), rows normalized by the
     reciprocal of column 64.
  4. Output stored [S, C] fp32; host transposes to [C, H, W].
"""

import numpy as np

import concourse.bass as bass
import concourse.tile as tile
from concourse import bacc, mybir
from concourse.bass_utils import run_bass_kernel_spmd

F32 = mybir.dt.float32
BF16 = mybir.dt.bfloat16
AF = mybir.ActivationFunctionType
ALU = mybir.AluOpType

S = 1024          # sequence length (32*32)
CIN = 256         # input channels (Cq = Ck)
C = 512           # projection channels
NH = 8            # heads
DH = 64           # head dim
HW = 32           # spatial H = W
N_CORES = 8


def _build_module():
    nc = bacc.Bacc("TRN2", target_bir_lowering=False)

    vw_d = nc.dram_tensor("vw", [3 * C, CIN], F32, kind="ExternalInput").ap()
    qk_d = nc.dram_tensor("qk", [2 * CIN, S], BF16, kind="ExternalInput").ap()
    gb_d = nc.dram_tensor("gb", [5, C], F32, kind="ExternalInput").ap()
    bv_d = nc.dram_tensor("bv", [C], F32, kind="ExternalInput").ap()
    msk_d = nc.dram_tensor("msk", [128, 256], BF16, kind="ExternalInput").ap()
    o_d = nc.dram_tensor("o", [S, C], F32, kind="ExternalOutput").ap()

    with tile.TileContext(nc) as tc:
        with (
            tc.tile_pool(name="const", bufs=1) as const,
            tc.tile_pool(name="persist", bufs=1) as persist,
            tc.tile_pool(name="wtmp", bufs=2) as wtmp,
            tc.tile_pool(name="smalls", bufs=4) as smalls,
        ):
            # ---- merged input DMAs, spread across the two rings
            msk_sb = const.tile([128, 256], BF16, name="msk_sb")
            nc.sync.dma_start(out=msk_sb, in_=msk_d)
            eye_bf = msk_sb[:, 0:128]
            triu = msk_sb[:, 128:256]
            # vw[512t + 128c + p, i] -> vtiles[t][p, c, i]
            vtiles = []
            for t_i in range(3):
                vt_t = persist.tile([128, 4, CIN], F32, tag=f"vtile{t_i}",
                                    name=f"vtile{t_i}")
                nc.sync.dma_start(
                    out=vt_t,
                    in_=vw_d[512 * t_i:512 * (t_i + 1), :].rearrange(
                        "(n p) i -> p n i", p=128))
                vtiles.append(vt_t)
            qhalves = []
            for half in range(2):
                h_t = persist.tile([128, 2, S], BF16, tag=f"qk{half}",
                                   name=f"qk{half}")
                nc.gpsimd.dma_start(
                    out=h_t,
                    in_=qk_d[256 * half:256 * (half + 1), :].rearrange(
                        "(n p) i -> p n i", p=128))
                qhalves.append(h_t)
            qT = [qhalves[0][:, 0, :], qhalves[0][:, 1, :]]
            kTt = [qhalves[1][:, 0, :], qhalves[1][:, 1, :]]
            gb_sb = const.tile([128, 20], F32, name="gb_sb")
            nc.sync.dma_start(out=gb_sb,
                              in_=gb_d.rearrange("n (c p) -> p (n c)", p=128))
            g_sbs = [gb_sb[:, 0:4], gb_sb[:, 4:8], gb_sb[:, 8:12]]
            bq_sb = gb_sb[:, 12:16]
            bk_sb = gb_sb[:, 16:20]
            bvb = const.tile([128, C], F32, name="bvb")
            nc.gpsimd.dma_start(
                out=bvb,
                in_=bass.AP(tensor=bv_d.tensor, offset=bv_d.offset,
                            ap=[[0, 128]] + list(bv_d.ap)),
            )

            wT = []       # wT[t][m]: [128, 512] bf16 (v.T; scaled only for t=2)
            scales = []   # per-weight [128, 4] fp32 scale tiles
            QT, KT, VP = [], [], []
            OUT = [persist.tile([128, C], F32, tag=f"OUT{i}", name=f"OUT{i}")
                   for i in range(8)]

            with tc.tile_pool(name="psWp", bufs=1, space="PSUM") as psWp:
                # ---- weight prep
                for t_i in range(3):
                    scale_sb = const.tile([128, 4], F32, name=f"scale_sb{t_i}")
                    scales.append(scale_sb)
                    wp = [
                        psWp.tile([128, 512], F32, tag=f"wp{m}", bufs=2,
                                  name=f"wp{m}_{t_i}")
                        for m in range(2)
                    ]
                    for c in range(4):
                        vt = vtiles[t_i][:, c, :]
                        vt_bf = wtmp.tile([128, CIN], BF16, tag="vtb",
                                          name=f"vtb{t_i}_{c}")
                        nc.vector.tensor_copy(out=vt_bf, in_=vt)
                        # weight-norm scale (fp32, off the critical path for q/k)
                        sqv = wtmp.tile([128, CIN], F32, tag="sqv", name=f"sqv{t_i}_{c}")
                        ssum = smalls.tile([128, 1], F32, tag="ssum", name=f"ssum{t_i}_{c}")
                        nc.scalar.activation(out=sqv, in_=vt, func=AF.Square,
                                             accum_out=ssum)
                        snorm = smalls.tile([128, 1], F32, tag="snorm",
                                            name=f"snorm{t_i}_{c}")
                        nc.scalar.activation(out=snorm, in_=ssum, func=AF.Sqrt)
                        rs = smalls.tile([128, 1], F32, tag="rs", name=f"rs{t_i}_{c}")
                        nc.vector.reciprocal(rs, snorm)
                        nc.vector.tensor_mul(scale_sb[:, c:c + 1], rs,
                                             g_sbs[t_i][:, c:c + 1])
                        if t_i == 2:
                            # V: channels end up on the free dim, so scale must
                            # be baked into the transposed weight itself.
                            rhs_t = wtmp.tile([128, 128], BF16, tag="diag",
                                              name=f"diag{t_i}_{c}")
                            nc.vector.tensor_scalar_mul(out=rhs_t, in0=eye_bf,
                                                        scalar1=scale_sb[:, c:c + 1])
                        else:
                            rhs_t = eye_bf
                        # wp[m][:, 128c:+128] = v_c[:, 128m:+128].T (@ diag)
                        for m in range(2):
                            nc.tensor.matmul(
                                wp[m][:, 128 * c:128 * (c + 1)],
                                lhsT=vt_bf[:, 128 * m:128 * (m + 1)],
                                rhs=rhs_t,
                                start=True, stop=True,
                            )
                    pair = []
                    for m in range(2):
                        wTm = persist.tile([128, C], BF16, tag=f"wT{t_i}_{m}",
                                           name=f"wT{t_i}_{m}")
                        nc.vector.tensor_copy(out=wTm, in_=wp[m])
                        pair.append(wTm)
                    wT.append(pair)

            with tc.tile_pool(name="psW", bufs=1, space="PSUM") as psW:
                for ct in range(4):
                    QT.append(persist.tile([128, S], BF16, tag=f"QT{ct}", name=f"QT{ct}"))
                    KT.append(persist.tile([128, S], BF16, tag=f"KT{ct}", name=f"KT{ct}"))
                for st in range(8):
                    VP.append(persist.tile([128, NH * 65], BF16, tag=f"VP{st}",
                                           name=f"VP{st}"))

                def emit_proj_group(ct, g):
                    # g in 0..3 -> (q/k, n-half)
                    dst, wpair, src, scale_sb, b_sb, pnm = (
                        (QT, wT[0], qT, scales[0], bq_sb, "q"),
                        (KT, wT[1], kTt, scales[1], bk_sb, "k"),
                    )[g // 2]
                    n = g % 2
                    pp = psW.tile([128, 512], F32, tag="pp", bufs=2,
                                  name=f"pp{pnm}{ct}_{n}")
                    for kc in range(2):
                        nc.tensor.matmul(
                            pp,
                            lhsT=wpair[kc][:, 128 * ct:128 * (ct + 1)],
                            rhs=src[kc][:, 512 * n:512 * (n + 1)],
                            start=(kc == 0), stop=(kc == 1),
                        )
                    # fused weight-norm scale + bias epilogue (on DVE: the
                    # ACT variant thrashes activation table sets against Exp)
                    nc.vector.tensor_scalar(
                        out=dst[ct][:, 512 * n:512 * (n + 1)],
                        in0=pp,
                        scalar1=scale_sb[:, ct:ct + 1],
                        scalar2=b_sb[:, ct:ct + 1],
                        op0=ALU.mult, op1=ALU.add,
                    )

                def emit_proj(ct):
                    for g in range(4):
                        emit_proj_group(ct, g)

                def emit_v(st):
                    vp = VP[st]
                    ppv = psW.tile([128, 512], F32, tag="pp", bufs=2, name=f"ppv{st}")
                    for kc in range(2):
                        nc.tensor.matmul(
                            ppv,
                            lhsT=kTt[kc][:, 128 * st:128 * (st + 1)],
                            rhs=wT[2][kc],
                            start=(kc == 0), stop=(kc == 1),
                        )
                    vp3 = vp.rearrange("p (h c) -> p h c", c=65)
                    nc.gpsimd.memset(vp3[:, :, 64:65], 1.0)
                    nc.vector.tensor_add(
                        vp3[:, :, 0:64],
                        ppv.rearrange("p (h c) -> p h c", c=64),
                        bvb.rearrange("p (h c) -> p h c", c=64),
                    )

                with (
                    tc.tile_pool(name="psL", bufs=1, space="PSUM") as psL,
                    tc.tile_pool(name="psPV", bufs=2, space="PSUM") as psPV,
                    tc.tile_pool(name="epool", bufs=2) as epool,
                ):
                    def emit_L(a2, j, eTs):
                        # j >= 4: two consecutive j's share one psum tile and
                        # one exp per head (ACT per-op overhead is 352 cycles)
                        js = [j] if j < 4 else [j, j + 1]
                        njs_ = [S - 128 * jj for jj in js]
                        w = sum(njs_)
                        e = epool.tile([128, 2 * w], BF16, tag=f"e_{j}",
                                       name=f"e_{a2}_{j}")
                        offs = []   # per j in js: (off_h0, off_h1)
                        o = 0
                        for nj_ in njs_:
                            offs.append((o, w + o))
                            o += nj_
                        for jj, (o0, _o1) in zip(js, offs):
                            eTs.append((e, offs[js.index(jj)]))
                        for hi in range(2):
                            p0 = 64 * hi
                            lt = psL.tile([128, w], F32, tag=f"lt{hi}",
                                          name=f"lt{hi}_{a2}_{j}")
                            base = 0
                            for jj, nj_ in zip(js, njs_):
                                for c0 in range(0, nj_, 512):
                                    cw = min(512, nj_ - c0)
                                    nc.tensor.matmul(
                                        lt[:, base + c0:base + c0 + cw],
                                        lhsT=KT[a2][p0:p0 + 64,
                                                    128 * jj:128 * jj + 128],
                                        rhs=QT[a2][p0:p0 + 64,
                                                   128 * jj + c0:128 * jj + c0 + cw],
                                        start=True, stop=True,
                                    )
                                base += nj_
                            nc.scalar.activation(
                                out=e[:, hi * w:hi * w + w], in_=lt,
                                func=AF.Exp, scale=0.125)
                            for (o0, o1) in offs:
                                off = o0 if hi == 0 else o1
                                nc.vector.tensor_mul(
                                    e[:, off:off + 128],
                                    e[:, off:off + 128], triu)

                    def emit_PV(a2, i, eTs):
                        # both heads accumulate into one 1-bank psum tile
                        po = psPV.tile([128, 130], F32, tag="po",
                                       name=f"po_{a2}_{i}")
                        for hi in range(2):
                            hh = 2 * a2 + hi
                            for jj in range(i + 1):
                                et, (o0, o1) = eTs[jj]
                                base = (o0, o1)[hi] + 128 * (i - jj)
                                nc.tensor.matmul(
                                    po[:, 65 * hi:65 * hi + 65],
                                    lhsT=et[:, base:base + 128],
                                    rhs=VP[jj][:, 65 * hh:65 * hh + 65],
                                    start=(jj == 0), stop=(jj == i),
                                )
                        r = smalls.tile([128, 2], F32, tag="r",
                                        name=f"r{a2}_{i}")
                        nc.vector.reciprocal(
                            r, po.rearrange("p (g x) -> p g x", g=2)[:, :, 64:65])
                        for hi in range(2):
                            hh = 2 * a2 + hi
                            nc.vector.tensor_scalar_mul(
                                out=OUT[i][:, 64 * hh:64 * hh + 64],
                                in0=po[:, 65 * hi:65 * hi + 64],
                                scalar1=r[:, hi:hi + 1],
                            )

                    prev_eTs = None
                    for a2 in range(4):
                        emit_proj(a2)
                        if a2 == 1:
                            for st in range(8):
                                emit_v(st)
                        eTs = []
                        for j in range(8):
                            if j not in (5, 7):
                                emit_L(a2, j, eTs)
                            if prev_eTs is not None:
                                emit_PV(a2 - 1, 7 - j, prev_eTs)
                            if a2 == 3:
                                emit_PV(3, j, eTs)
                        prev_eTs = eTs
                    # query row 0 attends to nothing: reference zeroes it
                    nc.vector.memset(OUT[0][0:1, :], 0.0)
            rings = [nc.sync, nc.gpsimd, nc.scalar]
            for i in range(8):
                rings[i % 3].dma_start(out=o_d[128 * i:128 * (i + 1), :],
                                       in_=OUT[i])
    nc.compile()
    return nc


_CACHE = {}


def _get_module():
    if "nc" not in _CACHE:
        _CACHE["nc"] = _build_module()
    return _CACHE["nc"]


def _in_maps(inputs):
    import ml_dtypes

    q = np.asarray(inputs["query"], dtype=np.float32)
    k = np.asarray(inputs["key"], dtype=np.float32)
    B = q.shape[0]
    assert B == N_CORES
    vw = np.ascontiguousarray(np.concatenate(
        [np.asarray(inputs[f"v{nm}"], np.float32) for nm in ("q", "k", "v")], axis=0))
    gb = np.ascontiguousarray(np.stack(
        [np.asarray(inputs["gq"], np.float32),
         np.asarray(inputs["gk"], np.float32),
         np.asarray(inputs["gv"], np.float32),
         np.asarray(inputs["bq"], np.float32),
         np.asarray(inputs["bk"], np.float32)]))
    bv = np.ascontiguousarray(np.asarray(inputs["bv"], np.float32))
    eye = np.eye(128, dtype=ml_dtypes.bfloat16)
    triu = np.triu(np.ones((128, 128), np.float32), k=1).astype(ml_dtypes.bfloat16)
    msk = np.ascontiguousarray(np.concatenate([eye, triu], axis=1))
    shared = {"vw": vw, "gb": gb, "bv": bv, "msk": msk}
    maps = []
    for b in range(B):
        m = dict(shared)
        m["qk"] = np.ascontiguousarray(np.concatenate(
            [q[b].reshape(CIN, S), k[b].reshape(CIN, S)], axis=0
        ).astype(ml_dtypes.bfloat16))
        maps.append(m)
    return maps


def _gather(results):
    outs = []
    for b in range(N_CORES):
        o = results[b]["o"]                       # [S, C]
        outs.append(np.ascontiguousarray(o.T).reshape(C, HW, HW))
    return np.stack(outs).astype(np.float32)      # [B, C, H, W]


def run(inputs, **kw):
    """Run on hardware; returns (full_output, BassKernelResults)."""
    nc = _get_module()
    res = run_bass_kernel_spmd(nc, _in_maps(inputs), list(range(N_CORES)), **kw)
    return _gather(res.results), res


def kernel(**inputs):
    out, _ = run(inputs)
    return out

